# revision 10
# baseline (speedup 1.0000x reference)
"""Bass/Trainium2 kernel for nn_GraphSToV (gnn_message_passing).

Computes, for full inputs:
  scalar_features [B=8, N=128, F=128] f32
  distances       [B=8, N=128, N=128, C=3] f32
  W               [2F=256, K=128] f32
  b               [1, K=128] f32
Output:
  vector_features [B, N, N, C, K] f32
  = (h_i @ W1 + h_j @ W2 + b)[b,i,j,k] * distances[b,i,j,c]

Sharding: data-parallel over batch B across 8 NeuronCores (1 batch each).
Host-side input staging (pure layout/dtype prep, no arithmetic):
  hT16   [F, N]       fp16  h transposed
  W1x4   [F, 4K]      fp16  W1 replicated 4x (dense matmul rhs streams
                            ~2x faster than a stride-0 replicated AP)
  W2_16  [F, K], b16 [1, K] fp16
  dist2  [N, C*N*2]   fp16  dist2[i, c*2N + j*2 + d] = dist[i,j,c],
                            each value duplicated (d in {0,1})

Per-core dataflow (partition dim = i everywhere):
  s2b  = h @ W2 + b                      (PE fp16, bias via rank-1 ones x b)
  s2b flattened so each 4-row block is a free-dim slice readable as a
    rank-1 matmul rhs (quad-flat: partitions 0/32/64 hold 32 rows each).
  for each group g of JG=8 j's:
    pair_psum[:, half] = h @ W1x4 + ones x s2b[j-block]  (PE, PSUM accum)
    pair_sb (fp16)     = copy(pair_psum)                 (ACT, f32->fp16)
    for c in 0..2:  ONE DVE tensor_tensor per c:
      out_sb[i,(j,c,k)] = pair_sb[i,(j,k)] * dist2[i,(c,j,dup)]
      via APs  dst [[384,8],[2,64],[1,2]]  in0 [[128,8],[2,64],[1,2]]
               in1 [[2,8],[0,64],[1,2]]
      The x2 host-side duplication makes the broadcast operand's
      innermost AP dim [stride=1, size=2], keeping the DVE in its 2x
      packed perf mode (a stride-0 innermost would force 1x). All
      three shapes are (8,64,2) so operand APs match exactly.
    DMA out_sb -> DRAM fp16, alternating sync/gpsimd rings (the Pool
    engine runs no elementwise work: its HW tensor_scalar path is
    ~20x slower per element than the DVE).
  Host upcasts fp16 -> f32. Max rel err ~1.1e-3 vs the 2e-2 gate;
  fp16 halves HBM write traffic (the roofline term: 12.6 MB/core out).
"""

import numpy as np
from contextlib import ExitStack

import concourse.bass as bass
import concourse.bacc as bacc
import concourse.mybir as mybir
import concourse.tile as tile
from concourse.bass_utils import run_bass_kernel_spmd

B, N, F, C, K = 8, 128, 128, 3, 128
JG = 8            # j's per group (one out-DMA per group)
NG = N // JG      # number of groups
F32 = mybir.dt.float32
F16 = mybir.dt.float16

# Out-DMA rings: sync + scalar are the HWDGE queues (gpsimd's queue is
# software-DGE — descriptor generation on the Q7 cores is slower). Each
# group's write is split in half across both rings so two HW queues keep
# the 16 DMA engines fed through per-DMA descriptor-gen/semaphore gaps.
OUT_RINGS = ("sync", "scalar")

_CACHE = {}


def _build_nc(reps=1):
    # Bacc (not raw Bass): its finalize() runs move_matmul_waits_to_ldweights
    # + generate_event_semaphores, which legalize multi-wait instructions for
    # the TRN2 one-wait-per-instruction ISA constraint.
    nc = bacc.Bacc()
    hT_d = nc.declare_dram_parameter("hT16", [F, N], F16, isOutput=False)
    W1x4_d = nc.declare_dram_parameter("W1x4", [F, 4 * K], F16, isOutput=False)
    W2_d = nc.declare_dram_parameter("W2_16", [F, K], F16, isOutput=False)
    b_d = nc.declare_dram_parameter("b16", [1, K], F16, isOutput=False)
    dist2_d = nc.declare_dram_parameter("dist2", [N, C * N * 2], F16, isOutput=False)
    out_d = nc.declare_dram_parameter("out", [N, N * C * K], F16, isOutput=True)

    with tile.TileContext(nc) as tc, ExitStack() as ctx:
        const = ctx.enter_context(tc.tile_pool(name="const", bufs=1))
        psum_setup = ctx.enter_context(tc.tile_pool(name="psum_setup", bufs=1, space="PSUM"))
        psum_pair = ctx.enter_context(tc.tile_pool(name="psum_pair", bufs=3, space="PSUM"))
        sb_pair = ctx.enter_context(tc.tile_pool(name="sb_pair", bufs=4))
        sb_out = ctx.enter_context(tc.tile_pool(name="sb_out", bufs=6))

        # Inputs needed by the s2b chain first; dist2/W1x4 can land later.
        hT_mm = const.tile([F, N], F16)
        nc.sync.dma_start(hT_mm[:], hT_d[:])
        W2_16 = const.tile([F, K], F16)
        nc.scalar.dma_start(W2_16[:], W2_d[:])
        b16 = const.tile([1, K], F16)
        nc.scalar.dma_start(b16[:], b_d[:])
        dist2 = const.tile([N, C * N * 2], F16)
        nc.gpsimd.dma_start(dist2[:], dist2_d[:])
        W1x4_t = const.tile([F, 4 * K], F16)
        nc.gpsimd.dma_start(W1x4_t[:], W1x4_d[:])

        # All-ones fp16, one row per partition: rank-1 lhsT must share its
        # base partition with the half-resident rhs slice of s2b_flat.
        ones16 = const.tile([N, N], F16)
        nc.vector.memset(ones16[:], 1.0)

        # s2b = h @ W2 + bias (bias broadcast via rank-1 ones x b)
        s2b_ps = psum_setup.tile([N, K], F32, tag="s2b_ps")
        nc.tensor.matmul(s2b_ps[:], hT_mm[:], W2_16[:], start=True, stop=False)
        nc.tensor.matmul(s2b_ps[:], ones16[0:1, :], b16[:], start=False, stop=True)
        s2b16 = const.tile([N, K], F16)
        nc.scalar.copy(s2b16[:], s2b_ps[:])

        # Flatten s2b rows so row j is a free-dim slice usable as a matmul
        # rhs (engine APs may only base at partition 0/32/64, not 96).
        # Partition 32q holds rows 32q..32q+31; quadrant 3 lands at
        # partition 64 of a second tile. Single-stage 256B-run DMAs: the
        # sub-512B 2x latency penalty costs ~50ns on 8KB quadrants, far
        # less than the extra serial DMA hop (~1.6us) a staged copy pays.
        s2b_flatA = const.tile([N, (N // 4) * K], F16)
        nc.sync.dma_start(s2b_flatA[0:3 * 32:32, :], s2b16[0:96, :])
        s2b_flatB = const.tile([N, (N // 4) * K], F16)
        nc.scalar.dma_start(s2b_flatB[64:65, :], s2b16[96:N, :])

        def rank1_ops(j0):
            if j0 < 96:
                q = j0 // 32
                return (ones16[32 * q:32 * q + 1, :],
                        s2b_flatA[32 * q:32 * q + 1,
                                  (j0 - 32 * q) * K:(j0 - 32 * q + 4) * K])
            return (ones16[64:65, :],
                    s2b_flatB[64:65, (j0 - 96) * K:(j0 - 96 + 4) * K])

        for g in range(NG * reps):
            g = g % NG
            pp = psum_pair.tile([N, 2 * 512], F32, tag="pp")
            # Both W1 halves first (one weight load), then both rank-1s
            # (shared ones16 row: j0 and j0+4 always sit in one quadrant).
            nc.tensor.matmul(pp[:, 0:512], hT_mm[:], W1x4_t[:],
                             start=True, stop=False)
            nc.tensor.matmul(pp[:, 512:1024], hT_mm[:], W1x4_t[:],
                             start=True, stop=False)
            for half in range(2):
                j0 = g * JG + half * 4
                sl = slice(half * 512, (half + 1) * 512)
                lhs1, rhs1 = rank1_ops(j0)
                nc.tensor.matmul(pp[:, sl], lhs1, rhs1,
                                 start=False, stop=True)
            pair_sb = sb_pair.tile([N, JG * K], F16, tag="pair")
            nc.scalar.copy(pair_sb[:], pp[:])

            out_sb = sb_out.tile([N, JG * C * K], F16, tag="out")
            # out_sb[i, (j,c,k)] = pair_sb[i, (j,k)] * dist[i, (j,c)]:
            # one dense 2x-mode tensor_tensor per c over all 8 j's.
            for c in range(C):
                dst = bass.AP(out_sb[:].tensor, c * K,
                              [[JG * C * K, N], [C * K, JG], [2, K // 2], [1, 2]])
                in0 = bass.AP(pair_sb[:].tensor, 0,
                              [[JG * K, N], [K, JG], [2, K // 2], [1, 2]])
                in1 = bass.AP(dist2[:].tensor, c * 2 * N + g * JG * 2,
                              [[C * N * 2, N], [2, JG], [0, K // 2], [1, 2]])
                nc.vector.tensor_mul(dst, in0, in1)

            half_w = JG * C * K // 2
            base = g * JG * C * K
            nc.sync.dma_start(
                out_d[:, base:base + half_w], out_sb[:, 0:half_w])
            nc.scalar.dma_start(
                out_d[:, base + half_w:base + 2 * half_w],
                out_sb[:, half_w:2 * half_w])
    nc.finalize()
    return nc


def _core_inputs(scalar_features, distances, W, b, core):
    h16 = np.asarray(scalar_features[core], dtype=np.float16)
    W16 = np.asarray(W, dtype=np.float16)
    # dist2[i, c*2N + j*2 + d] = dist[i, j, c]
    d16 = np.asarray(distances[core], dtype=np.float16)       # [N, N, C]
    dist2 = np.repeat(d16.transpose(0, 2, 1).reshape(N, C * N), 2, axis=1)
    return {
        "hT16": np.ascontiguousarray(h16.T),
        "W1x4": np.ascontiguousarray(np.tile(W16[:F], (1, 4))),
        "W2_16": np.ascontiguousarray(W16[F:]),
        "b16": np.ascontiguousarray(np.asarray(b).reshape(1, K).astype(np.float16)),
        "dist2": np.ascontiguousarray(dist2),
    }


def _assemble_core_output(out_raw):
    return np.asarray(out_raw).astype(np.float32).reshape(N, N, C, K)


def _run(scalar_features, distances, W, b, trace=False, reps=1):
    key = ("nc", reps, OUT_RINGS)
    if key not in _CACHE:
        _CACHE[key] = _build_nc(reps)
    nc = _CACHE[key]
    in_maps = [_core_inputs(scalar_features, distances, W, b, i) for i in range(B)]
    r = run_bass_kernel_spmd(nc, in_maps, list(range(B)), trace=trace)
    out = np.stack([_assemble_core_output(r.results[i]["out"]) for i in range(B)])
    return out, r


def kernel(scalar_features, distances, W, b):
    out, _ = _run(scalar_features, distances, W, b, trace=False)
    return out


# revision 16
# speedup vs baseline: 1.1558x; 1.1558x over previous
"""Bass/Trainium2 kernel for nn_GraphSToV (gnn_message_passing).

Computes, for full inputs:
  scalar_features [B=8, N=128, F=128] f32
  distances       [B=8, N=128, N=128, C=3] f32
  W               [2F=256, K=128] f32
  b               [1, K=128] f32
Output:
  vector_features [B, N, N, C, K] f32
  = (h_i @ W1 + h_j @ W2 + b)[b,i,j,k] * distances[b,i,j,c]

Sharding: data-parallel over batch B across 8 NeuronCores (1 batch each).
Host-side input staging (pure layout/dtype prep, no arithmetic):
  hT16   [F, N]       fp16  h transposed
  W1x4   [F, 4K]      fp16  W1 replicated 4x (dense matmul rhs streams
                            ~2x faster than a stride-0 replicated AP)
  W2_16  [F, K], b16 [1, K] fp16
  dist2  [N, C*N*2]   fp16  dist2[i, c*2N + j*2 + d] = dist[i,j,c],
                            each value duplicated (d in {0,1})

Per-core dataflow (partition dim = i everywhere):
  s2b  = h @ W2 + b                      (PE fp16, bias via rank-1 ones x b)
  s2b flattened so each 4-row block is a free-dim slice readable as a
    rank-1 matmul rhs (quad-flat: partitions 0/32/64 hold 32 rows each).
  for each group g of JG=8 j's:
    pair_psum[:, half] = h @ W1x4 + ones x s2b[j-block]  (PE, PSUM accum)
    pair_sb (fp16)     = copy(pair_psum)                 (ACT, f32->fp16)
    for c in 0..2:  ONE DVE tensor_tensor per c:
      out_sb[i,(j,c,k)] = pair_sb[i,(j,k)] * dist2[i,(c,j,dup)]
      via APs  dst [[384,8],[2,64],[1,2]]  in0 [[128,8],[2,64],[1,2]]
               in1 [[2,8],[0,64],[1,2]]
      The x2 host-side duplication makes the broadcast operand's
      innermost AP dim [stride=1, size=2], keeping the DVE in its 2x
      packed perf mode (a stride-0 innermost would force 1x). All
      three shapes are (8,64,2) so operand APs match exactly.
    DMA out_sb -> DRAM fp16, alternating sync/gpsimd rings (the Pool
    engine runs no elementwise work: its HW tensor_scalar path is
    ~20x slower per element than the DVE).
  Host upcasts fp16 -> f32. Max rel err ~1.1e-3 vs the 2e-2 gate;
  fp16 halves HBM write traffic (the roofline term: 12.6 MB/core out).
"""

import numpy as np
from contextlib import ExitStack

import concourse.bass as bass
import concourse.bacc as bacc
import concourse.mybir as mybir
import concourse.tile as tile
from concourse.bass_utils import run_bass_kernel_spmd

B, N, F, C, K = 8, 128, 128, 3, 128
JG = 8            # j's per group (one out-DMA per group)
NG = N // JG      # number of groups
F32 = mybir.dt.float32
F16 = mybir.dt.float16

# Out-DMA rings: sync + scalar are the HWDGE queues (gpsimd's queue is
# software-DGE — descriptor generation on the Q7 cores is slower). Each
# group's write is split in half across both rings so two HW queues keep
# the 16 DMA engines fed through per-DMA descriptor-gen/semaphore gaps.
OUT_RINGS = ("sync", "scalar")

_CACHE = {}


def _build_nc(reps=1):
    # Bacc (not raw Bass): its finalize() runs move_matmul_waits_to_ldweights
    # + generate_event_semaphores, which legalize multi-wait instructions for
    # the TRN2 one-wait-per-instruction ISA constraint.
    nc = bacc.Bacc()
    # wcat packs hT | W1x4 | W2 | b (b in row 0 of its block) so one DMA +
    # one semaphore hop unblocks every PE dependency.
    WCAT = N + 4 * K + K + K
    wcat_d = nc.declare_dram_parameter("wcat", [F, WCAT], F16, isOutput=False)
    dist2_d = nc.declare_dram_parameter("dist2", [N, C * N * 2], F16, isOutput=False)
    out_d = nc.declare_dram_parameter("out", [N, N * C * K], F16, isOutput=True)

    with tile.TileContext(nc) as tc, ExitStack() as ctx:
        const = ctx.enter_context(tc.tile_pool(name="const", bufs=1))
        psum_setup = ctx.enter_context(tc.tile_pool(name="psum_setup", bufs=1, space="PSUM"))
        psum_pair = ctx.enter_context(tc.tile_pool(name="psum_pair", bufs=3, space="PSUM"))
        sb_pair = ctx.enter_context(tc.tile_pool(name="sb_pair", bufs=4))
        sb_out = ctx.enter_context(tc.tile_pool(name="sb_out", bufs=6))

        wcat = const.tile([F, WCAT], F16)
        nc.sync.dma_start(wcat[:], wcat_d[:])
        hT_mm = wcat[:, 0:N]
        W1x4_t = wcat[:, N:N + 4 * K]
        W2_16 = wcat[:, N + 4 * K:N + 5 * K]
        b16 = wcat[0:1, N + 5 * K:N + 6 * K]
        dist2 = const.tile([N, C * N * 2], F16)
        nc.scalar.dma_start(dist2[:], dist2_d[:])

        # All-ones fp16, one row per partition: rank-1 lhsT must share its
        # base partition with the half-resident rhs slice of s2b_flat.
        ones16 = const.tile([N, N], F16)
        nc.vector.memset(ones16[:], 1.0)

        # s2b = h @ W2 + bias (bias broadcast via rank-1 ones x b)
        s2b_ps = psum_setup.tile([N, K], F32, tag="s2b_ps")
        nc.tensor.matmul(s2b_ps[:], hT_mm, W2_16, start=True, stop=False)
        nc.tensor.matmul(s2b_ps[:], ones16[0:1, :], b16, start=False, stop=True)
        s2b16 = const.tile([N, K], F16)

        # Flatten s2b rows so row j is a free-dim slice usable as a matmul
        # rhs (engine APs may only base at partition 0/32/64, not 96).
        # Partition 32q holds rows 32q..32q+31; quadrant 3 lands at
        # partition 64 of a second tile. Single-stage 256B-run DMAs: the
        # sub-512B 2x latency penalty costs ~50ns on 8KB quadrants, far
        # less than an extra serial DMA hop. Quadrant 0 is evacuated and
        # flattened first so group 0's rank-1 unblocks ~2us earlier (the
        # tile framework tracks sub-tile ranges).
        s2b_flatA = const.tile([N, (N // 4) * K], F16)
        s2b_flatB = const.tile([N, (N // 4) * K], F16)
        nc.scalar.copy(s2b16[0:32, :], s2b_ps[0:32, :])
        nc.sync.dma_start(s2b_flatA[0:1, :], s2b16[0:32, :])
        nc.scalar.copy(s2b16[32:64, :], s2b_ps[32:64, :])
        nc.scalar.copy(s2b16[64:N, :], s2b_ps[64:N, :])
        nc.sync.dma_start(s2b_flatA[32:3 * 32:32, :], s2b16[32:96, :])
        nc.scalar.dma_start(s2b_flatB[64:65, :], s2b16[96:N, :])

        def rank1_ops(j0):
            if j0 < 96:
                q = j0 // 32
                return (ones16[32 * q:32 * q + 1, :],
                        s2b_flatA[32 * q:32 * q + 1,
                                  (j0 - 32 * q) * K:(j0 - 32 * q + 4) * K])
            return (ones16[64:65, :],
                    s2b_flatB[64:65, (j0 - 96) * K:(j0 - 96 + 4) * K])

        for g in range(NG * reps):
            g = g % NG
            pp = psum_pair.tile([N, 2 * 512], F32, tag="pp")
            # Both W1 halves first (one weight load), then both rank-1s
            # (shared ones16 row: j0 and j0+4 always sit in one quadrant).
            nc.tensor.matmul(pp[:, 0:512], hT_mm, W1x4_t,
                             start=True, stop=False)
            nc.tensor.matmul(pp[:, 512:1024], hT_mm, W1x4_t,
                             start=True, stop=False)
            for half in range(2):
                j0 = g * JG + half * 4
                sl = slice(half * 512, (half + 1) * 512)
                lhs1, rhs1 = rank1_ops(j0)
                nc.tensor.matmul(pp[:, sl], lhs1, rhs1,
                                 start=False, stop=True)
            pair_sb = sb_pair.tile([N, JG * K], F16, tag="pair")
            nc.scalar.copy(pair_sb[:], pp[:])

            out_sb = sb_out.tile([N, JG * C * K], F16, tag="out")
            # out_sb[i, (j,c,k)] = pair_sb[i, (j,k)] * dist[i, (j,c)]:
            # one dense 2x-mode tensor_tensor per c over all 8 j's.
            for c in range(C):
                dst = bass.AP(out_sb[:].tensor, c * K,
                              [[JG * C * K, N], [C * K, JG], [2, K // 2], [1, 2]])
                in0 = bass.AP(pair_sb[:].tensor, 0,
                              [[JG * K, N], [K, JG], [2, K // 2], [1, 2]])
                in1 = bass.AP(dist2[:].tensor, c * 2 * N + g * JG * 2,
                              [[C * N * 2, N], [2, JG], [0, K // 2], [1, 2]])
                nc.vector.tensor_mul(dst, in0, in1)

            half_w = JG * C * K // 2
            base = g * JG * C * K
            nc.sync.dma_start(
                out_d[:, base:base + half_w], out_sb[:, 0:half_w])
            nc.scalar.dma_start(
                out_d[:, base + half_w:base + 2 * half_w],
                out_sb[:, half_w:2 * half_w])
    nc.finalize()
    return nc


def _core_inputs(scalar_features, distances, W, b, core):
    h16 = np.asarray(scalar_features[core], dtype=np.float16)
    W16 = np.asarray(W, dtype=np.float16)
    b_blk = np.zeros((F, K), dtype=np.float16)
    b_blk[0] = np.asarray(b).reshape(K).astype(np.float16)
    wcat = np.concatenate(
        [h16.T, np.tile(W16[:F], (1, 4)), W16[F:], b_blk], axis=1)
    # dist2[i, c*2N + j*2 + d] = dist[i, j, c]
    d16 = np.asarray(distances[core], dtype=np.float16)       # [N, N, C]
    dist2 = np.repeat(d16.transpose(0, 2, 1).reshape(N, C * N), 2, axis=1)
    return {
        "wcat": np.ascontiguousarray(wcat),
        "dist2": np.ascontiguousarray(dist2),
    }


def _assemble_core_output(out_raw):
    return np.asarray(out_raw).astype(np.float32).reshape(N, N, C, K)


def _run(scalar_features, distances, W, b, trace=False, reps=1):
    key = ("nc", reps, OUT_RINGS)
    if key not in _CACHE:
        _CACHE[key] = _build_nc(reps)
    nc = _CACHE[key]
    in_maps = [_core_inputs(scalar_features, distances, W, b, i) for i in range(B)]
    r = run_bass_kernel_spmd(nc, in_maps, list(range(B)), trace=trace)
    out = np.stack([_assemble_core_output(r.results[i]["out"]) for i in range(B)])
    return out, r


def kernel(scalar_features, distances, W, b):
    out, _ = _run(scalar_features, distances, W, b, trace=False)
    return out
